# revision 11
# baseline (speedup 1.0000x reference)
"""Distributed multi-head attention kernel for one TRN2 chip (8 NeuronCores).

Problem: x[2,2048,1024] -> qkv proj (W_qkv[3072,1024], b_qkv) -> 16-head
attention (d_key=64) -> out proj (W_o[1024,1024], b_o).

Sharding: head tensor-parallel, 2 heads per core.  Everything on-device is
computed in transposed orientation so no transposes are ever needed:

  per core (heads h0=2c, h1=2c+1):
    qT/kT  [128, 4096]  (2 heads stacked on partitions; cols = b*2048+t)
        computed as  W_q_local @ x^T   (weights stationary)
    v      [128pos x 32tile x 130]  natural orientation (x^T stationary),
        with zero-weight/bias-1 "ones" columns at 64 and 129 so the ctx
        matmul's lhsT = [v_h | ones] produces the softmax denominator for
        free on psum partition 64.
    per (batch, head):  logitsT[keys,q] = kT_tile.T-as-lhsT @ qT  (K=64)
        E = exp(logits/8)      (no max subtraction: |logits/8| < ~2)
        ctxT_unnorm[64,q] + colsum[1,q] accumulated over key tiles in PSUM
        normalize by DMA-broadcast reciprocal of colsum
    AllToAll of ctxT (partition block j = q-column chunk j) -> each core
        gets full concatT[1024, its 512 rows] in global head order
    out rows [512c:512c+512] = concat_rows @ W_o^T + b_o   (ones-row bias
        matmul trick), host concatenates the 8 row blocks.

Matmul/compute dtype bf16 (f32 PSUM accumulation); rel-err gate is 2e-2.
"""

import sys

sys.path.insert(0, "/opt/trn_rl_repo")

import numpy as np
import ml_dtypes

import concourse.bass as bass
import concourse.tile as tile
from concourse import bacc, mybir
from concourse.bass_utils import run_bass_kernel_spmd

BF16 = mybir.dt.bfloat16
F32 = mybir.dt.float32
NPBF16 = ml_dtypes.bfloat16

D = 1024  # d_model
T = 2048  # seq len
B = 2  # batch
P = B * T  # 4096 total positions
H = 16  # total heads
DK = 64  # head dim
NCORES = 8
HL = H // NCORES  # 2 heads per core
KT = 128  # key tile
QT = 512  # q tile (psum free width)


def build_graph():
    nc = bacc.Bacc(
        "TRN2", target_bir_lowering=False, debug=False, num_devices=NCORES
    )

    # --- per-core external inputs (bf16 unless noted) ---
    xT = nc.declare_dram_parameter("xT", [D, P], BF16, isOutput=False)
    wqT = nc.declare_dram_parameter("wqT", [D, 128], BF16, isOutput=False)
    wkT = nc.declare_dram_parameter("wkT", [D, 128], BF16, isOutput=False)
    wvT = nc.declare_dram_parameter("wvT", [D, 130], BF16, isOutput=False)
    bq = nc.declare_dram_parameter("bq", [1, 128], BF16, isOutput=False)
    bk = nc.declare_dram_parameter("bk", [1, 128], BF16, isOutput=False)
    bv = nc.declare_dram_parameter("bv", [1, 130], BF16, isOutput=False)
    woT = nc.declare_dram_parameter("woT", [D, D], BF16, isOutput=False)
    bo = nc.declare_dram_parameter("bo", [1, D], BF16, isOutput=False)
    out = nc.declare_dram_parameter("out", [P // NCORES, D], F32, isOutput=True)

    with tile.TileContext(nc) as tc:
        with (
            tc.tile_pool(name="const", bufs=1) as const_pool,
            tc.tile_pool(name="xw", bufs=1) as xw_pool,
            tc.tile_pool(name="qkv", bufs=1) as qkv_pool,
            tc.tile_pool(name="et", bufs=2) as et_pool,
            tc.tile_pool(name="norm", bufs=2) as norm_pool,
            tc.tile_pool(name="ctxn", bufs=4) as ctxn_pool,
            tc.tile_pool(name="ow", bufs=2) as ow_pool,
            tc.tile_pool(name="obuf", bufs=2) as obuf_pool,
            tc.tile_pool(name="ps_mm", bufs=2, space="PSUM") as ps_mm,
            tc.tile_pool(name="ps_log", bufs=2, space="PSUM") as ps_log,
            tc.tile_pool(name="ps_ctx", bufs=2, space="PSUM") as ps_ctx,
            tc.tile_pool(name="dram", bufs=1, space="DRAM") as dram_pool,
        ):
            # --- load x^T and weights into SBUF ---
            x_sb = xw_pool.tile([128, 8, P], BF16)  # [part, ktile, pos]
            for kt in range(8):
                nc.sync.dma_start(out=x_sb[:, kt, :], in_=xT[kt * 128 : (kt + 1) * 128, :])
            wq_sb = xw_pool.tile([128, 8, 128], BF16)
            wk_sb = xw_pool.tile([128, 8, 128], BF16)
            wv_sb = xw_pool.tile([128, 8, 130], BF16)
            for kt in range(8):
                nc.sync.dma_start(out=wq_sb[:, kt, :], in_=wqT[kt * 128 : (kt + 1) * 128, :])
                nc.sync.dma_start(out=wk_sb[:, kt, :], in_=wkT[kt * 128 : (kt + 1) * 128, :])
                nc.sync.dma_start(out=wv_sb[:, kt, :], in_=wvT[kt * 128 : (kt + 1) * 128, :])
            bq_sb = const_pool.tile([1, 128], BF16)
            bk_sb = const_pool.tile([1, 128], BF16)
            bv_sb = const_pool.tile([1, 130], BF16)
            bo_sb = const_pool.tile([1, D], BF16)
            nc.sync.dma_start(out=bq_sb, in_=bq[:, :])
            nc.sync.dma_start(out=bk_sb, in_=bk[:, :])
            nc.sync.dma_start(out=bv_sb, in_=bv[:, :])
            nc.sync.dma_start(out=bo_sb, in_=bo[:, :])
            ones_sb = const_pool.tile([1, 128], BF16)
            nc.vector.memset(ones_sb, 1.0)
            ones512_sb = const_pool.tile([1, 512], BF16)
            nc.vector.memset(ones512_sb, 1.0)

            wo_sb = ow_pool.tile([128, 8, D], BF16)
            for kt in range(8):
                nc.sync.dma_start(out=wo_sb[:, kt, :], in_=woT[kt * 128 : (kt + 1) * 128, :])

            # --- QKV projection ---
            # qT/kT: [128 feat (2 heads), P]  = W @ x^T  (weights stationary)
            q_sb = qkv_pool.tile([128, P], BF16)
            k_sb = qkv_pool.tile([128, P], BF16)
            for p8 in range(8):
                sl = slice(p8 * 512, (p8 + 1) * 512)
                ps = ps_mm.tile([128, 512], F32, tag="mm")
                for kt in range(8):
                    nc.tensor.matmul(
                        out=ps,
                        lhsT=wq_sb[:, kt, :],
                        rhs=x_sb[:, kt, sl],
                        start=(kt == 0),
                        stop=False,
                    )
                nc.tensor.matmul(
                    out=ps, lhsT=bq_sb, rhs=ones512_sb, start=False, stop=True
                )
                nc.vector.tensor_copy(out=q_sb[:, sl], in_=ps)
                ps = ps_mm.tile([128, 512], F32, tag="mm")
                for kt in range(8):
                    nc.tensor.matmul(
                        out=ps,
                        lhsT=wk_sb[:, kt, :],
                        rhs=x_sb[:, kt, sl],
                        start=(kt == 0),
                        stop=False,
                    )
                nc.tensor.matmul(
                    out=ps, lhsT=bk_sb, rhs=ones512_sb, start=False, stop=True
                )
                nc.vector.tensor_copy(out=k_sb[:, sl], in_=ps)
            # v: natural orientation [pos, feat], x^T stationary.
            # columns 64 and 129 of wvT/bv are the "ones" columns.
            v_sb = qkv_pool.tile([128, 32, 130], BF16)
            for pt in range(32):
                ps = ps_mm.tile([128, 130], F32, tag="mm")
                for kt in range(8):
                    nc.tensor.matmul(
                        out=ps,
                        lhsT=x_sb[:, kt, pt * 128 : (pt + 1) * 128],
                        rhs=wv_sb[:, kt, :],
                        start=(kt == 0),
                        stop=False,
                    )
                nc.tensor.matmul(
                    out=ps,
                    lhsT=ones_sb[:, 0:128],
                    rhs=bv_sb,
                    start=False,
                    stop=True,
                )
                nc.vector.tensor_copy(out=v_sb[:, pt, :], in_=ps)

            # --- A2A bounce buffers ---
            cc_in = dram_pool.tile([NCORES * 128, 512], BF16)
            cc_out = dram_pool.tile([NCORES * 128, 512], BF16)

            # --- attention per (batch, head, q-half) ---
            for b in range(B):
                for h in range(HL):
                    po = DK * h  # partition offset of this head in q_sb/k_sb
                    co = b * T  # column (position) offset of this batch
                    for qh in range(2):
                        qco = co + qh * 1024
                        ps_c = [
                            ps_ctx.tile([65, 512], F32, tag="ctx", name=f"psc{qt}")
                            for qt in range(2)
                        ]
                        for kt in range(16):
                            ps_l = ps_log.tile([128, 1024], F32, tag="log")
                            for qt in range(2):
                                nc.tensor.matmul(
                                    out=ps_l[:, qt * 512 : (qt + 1) * 512],
                                    lhsT=k_sb[po : po + DK, co + kt * 128 : co + (kt + 1) * 128],
                                    rhs=q_sb[po : po + DK, qco + qt * 512 : qco + (qt + 1) * 512],
                                    start=True,
                                    stop=True,
                                )
                            et = et_pool.tile([128, 1024], BF16, tag="et")
                            nc.scalar.activation(
                                out=et, in_=ps_l,
                                func=mybir.ActivationFunctionType.Exp,
                                scale=0.125,
                            )
                            for qt in range(2):
                                nc.tensor.matmul(
                                    out=ps_c[qt],
                                    lhsT=v_sb[:, b * 16 + kt, 65 * h : 65 * h + 65],
                                    rhs=et[:, qt * 512 : (qt + 1) * 512],
                                    start=(kt == 0),
                                    stop=(kt == 15),
                                )
                        # normalize: ctxT[0:64] / colsum(row 64)
                        rs = norm_pool.tile([65, 1024], F32, tag="rsum")
                        for qt in range(2):
                            nc.vector.reciprocal(
                                out=rs[64:65, qt * 512 : (qt + 1) * 512],
                                in_=ps_c[qt][64:65, :],
                            )
                        rsd = dram_pool.tile([1, 1024], F32, tag="rsd", bufs=2)
                        nc.sync.dma_start(out=rsd, in_=rs[64:65, :])
                        rbc = norm_pool.tile([64, 1024], F32, tag="rbc")
                        nc.sync.dma_start(
                            out=rbc, in_=rsd.to_broadcast([64, 1024])
                        )
                        for qt in range(2):
                            ctxn = ctxn_pool.tile([64, 512], BF16, tag="cn")
                            nc.vector.tensor_mul(
                                out=ctxn,
                                in0=ps_c[qt][0:64, :],
                                in1=rbc[:, qt * 512 : (qt + 1) * 512],
                            )
                            j = (b * T + qh * 1024 + qt * 512) // 512
                            nc.sync.dma_start(
                                out=cc_in[j * 128 + DK * h : j * 128 + DK * h + DK, :],
                                in_=ctxn,
                            )

            # --- AllToAll: core j gets concatT[1024, rows 512j:512j+512] ---
            nc.gpsimd.collective_compute(
                "AllToAll",
                mybir.AluOpType.bypass,
                replica_groups=[list(range(NCORES))],
                ins=[cc_in[:].opt()],
                outs=[cc_out[:].opt()],
            )

            # --- output projection for my 512 rows ---
            for rt in range(4):
                lw = obuf_pool.tile([128, 8, 128], BF16, tag="lw")
                for kt in range(8):
                    nc.sync.dma_start(
                        out=lw[:, kt, :],
                        in_=cc_out[kt * 128 : (kt + 1) * 128, rt * 128 : (rt + 1) * 128],
                    )
                o_sb = obuf_pool.tile([128, D], F32, tag="ob")
                for nt in range(2):
                    ps = ps_mm.tile([128, 512], F32, tag="mm")
                    for kt in range(8):
                        nc.tensor.matmul(
                            out=ps,
                            lhsT=lw[:, kt, :],
                            rhs=wo_sb[:, kt, nt * 512 : (nt + 1) * 512],
                            start=(kt == 0),
                            stop=False,
                        )
                    nc.tensor.matmul(
                        out=ps,
                        lhsT=ones_sb,
                        rhs=bo_sb[:, nt * 512 : (nt + 1) * 512],
                        start=False,
                        stop=True,
                    )
                    nc.vector.tensor_copy(out=o_sb[:, nt * 512 : (nt + 1) * 512], in_=ps)
                nc.sync.dma_start(out=out[rt * 128 : (rt + 1) * 128, :], in_=o_sb)

    nc.compile()
    return nc


def make_in_maps(x, W_qkv, b_qkv, W_o, b_o):
    x = np.asarray(x, dtype=np.float32)
    W_qkv = np.asarray(W_qkv, dtype=np.float32)
    b_qkv = np.asarray(b_qkv, dtype=np.float32)
    W_o = np.asarray(W_o, dtype=np.float32)
    b_o = np.asarray(b_o, dtype=np.float32)

    xT = np.ascontiguousarray(x.reshape(P, D).T).astype(NPBF16)
    woT = np.ascontiguousarray(W_o.T).astype(NPBF16)
    bo = b_o.reshape(1, D).astype(NPBF16)

    in_maps = []
    for c in range(NCORES):
        r = slice(128 * c, 128 * c + 128)
        wq = W_qkv[0 * D :][r.start : r.stop]  # [128, 1024] q features
        wk = W_qkv[1 * D + 128 * c : 1 * D + 128 * c + 128]
        wv = W_qkv[2 * D + 128 * c : 2 * D + 128 * c + 128]
        wvT_pad = np.zeros((D, 130), dtype=np.float32)
        wvT_pad[:, 0:64] = wv[0:64].T
        wvT_pad[:, 65:129] = wv[64:128].T
        bv_pad = np.zeros((1, 130), dtype=np.float32)
        bv_pad[0, 0:64] = b_qkv[2 * D + 128 * c : 2 * D + 128 * c + 64]
        bv_pad[0, 64] = 1.0
        bv_pad[0, 65:129] = b_qkv[2 * D + 128 * c + 64 : 2 * D + 128 * c + 128]
        bv_pad[0, 129] = 1.0
        in_maps.append(
            {
                "xT": xT,
                "wqT": np.ascontiguousarray(wq.T).astype(NPBF16),
                "wkT": np.ascontiguousarray(wk.T).astype(NPBF16),
                "wvT": wvT_pad.astype(NPBF16),
                "bq": b_qkv[128 * c : 128 * c + 128].reshape(1, 128).astype(NPBF16),
                "bk": b_qkv[D + 128 * c : D + 128 * c + 128].reshape(1, 128).astype(NPBF16),
                "bv": bv_pad.astype(NPBF16),
                "woT": woT,
                "bo": bo,
            }
        )
    return in_maps


_CACHED_GRAPH = None


def kernel(x, W_qkv, b_qkv, W_o, b_o):
    global _CACHED_GRAPH
    if _CACHED_GRAPH is None:
        _CACHED_GRAPH = build_graph()
    nc = _CACHED_GRAPH
    in_maps = make_in_maps(x, W_qkv, b_qkv, W_o, b_o)
    res = run_bass_kernel_spmd(nc, in_maps, core_ids=list(range(NCORES)))
    outs = [res.results[c]["out"] for c in range(NCORES)]
    full = np.concatenate(outs, axis=0).astype(np.float32).reshape(B, T, D)
    return full
